# revision 39
# baseline (speedup 1.0000x reference)
"""Trainium2 Bass kernel: DepthSeparableConv2d (dw3x3 + BN + ReLU + map-cut,
pw 1x1 + BN + ReLU + map-cut), data-parallel over batch on 8 NeuronCores.

Host side folds all the small weight algebra (BN scales into conv weights,
pw transpose, biases, the dw-cut threshold) in numpy; the device kernel is
a pure streaming pipeline per core (4 images):

  - depthwise 3x3 conv as 9 diagonal-matmul "taps" on the TensorEngine
    (bf16 diag weights pre-scaled by the BN1 scale, bf16 activations, fp32
    PSUM accumulation); zero padding realized by AP sub-ranges + strided
    PSUM outputs, so the input DMA stays fully contiguous.
  - PSUM tiles are 2-bank [128,1024] pairs; each pair drains in ONE
    Scalar/Vector instruction (relu(psum+bias), bf16), and the dw map-cut
    stat is one VectorE XY-reduce straight from PSUM (keep = max(psum) >=
    4.0 - bias1, threshold folded on host) so the keep chain never waits
    on drains.  keep is folded into the pointwise lhsT halves.
  - pointwise 1x1: 2 chunks of 128 out-channels, bf16 matmuls, drain
    relu(psum+bias2) -> bf16 z per 448-px tile, output DMA per half-chunk.
  - pw map-cut is applied ON HOST (exact max >= 0.001 test in numpy);
    output DMA is bf16 (half traffic), host casts to fp32.
  - schedule: pw(n) is emitted BETWEEN pairs 1 and 2 of dw(n+1), so every
    keep chain and every pw burst is covered by depthwise matmuls; pw3's
    chain is covered by a few dummy matmuls.
"""

import numpy as np

B, C_IN, C_OUT, H, W = 32, 128, 256, 56, 56
N_CORES = 8
BPC = B // N_CORES          # images per core
HW = H * W                  # 3136
TILE_ROWS = 8               # output rows per 448-px sub-tile
NT = H // TILE_ROWS         # 7 sub-tiles per image
TN = TILE_ROWS * W          # 448 pixels per sub-tile
BN_EPS = 1e-5
DW_THRESH = 4.0
PW_THRESH = 0.001

# pairs of 448-px sub-tiles sharing one 2-bank PSUM tile
PAIRS = [(0, 1), (2, 3), (4, 5), (6,)]

# tap order: (0,0) first so the start=True matmul covers the full tile
TAPS = [(0, 0), (-1, 0), (1, 0), (0, -1), (0, 1),
        (-1, -1), (-1, 1), (1, -1), (1, 1)]

_CACHE = {}


def _build():
    import concourse.bacc as bacc
    import concourse.tile as tile
    import concourse.mybir as mybir

    f32 = mybir.dt.float32
    bf16 = mybir.dt.bfloat16
    Alu = mybir.AluOpType
    Act = mybir.ActivationFunctionType

    nc = bacc.Bacc("TRN2", target_bir_lowering=False, debug=False,
                   enable_asserts=False, num_devices=N_CORES)

    x_d = nc.dram_tensor("x", [BPC, C_IN, H, W], bf16, kind="ExternalInput").ap()
    dg_d = nc.dram_tensor("diags", [C_IN, 9, C_IN], bf16, kind="ExternalInput").ap()
    # bias1 / thr1 / bias2-lo / bias2-hi packed as one [C_IN, 4] tensor:
    # a [128,1] vector DMA costs 128 four-byte descriptors, so packing
    # quarters the descriptor load
    bv_d = nc.dram_tensor("biasv", [C_IN, 4], f32, kind="ExternalInput").ap()
    lw_d = nc.dram_tensor("lhsTb", [C_IN, C_OUT], bf16, kind="ExternalInput").ap()
    z_d = nc.dram_tensor("z", [BPC, C_OUT, H, W], bf16, kind="ExternalOutput").ap()

    with tile.TileContext(nc) as tc:
        with tc.tile_pool(name="const", bufs=1) as cp, \
             tc.tile_pool(name="xb", bufs=3) as xbp, \
             tc.tile_pool(name="y", bufs=3) as yp, \
             tc.tile_pool(name="z", bufs=4) as zp, \
             tc.tile_pool(name="small", bufs=8) as sp, \
             tc.tile_pool(name="dwps", bufs=2, space="PSUM") as dwps_pool, \
             tc.tile_pool(name="pwps", bufs=2, space="PSUM") as pwps_pool:

            # ---- startup DMAs: diag weights, then whole image 0, on the
            # sync queue.  One DMA per tensor: the DMA engines are
            # descriptor-rate-bound (~290ns per per-partition line), so
            # fewer, fatter descriptors win ----
            dgt = cp.tile([128, 9 * 128], bf16)
            xb0 = xbp.tile([128, H, W], bf16, name="xbt")
            # dgt on the gpsimd queue, image 0 on the sync queue: their
            # descriptors interleave across the 16 DMA engines in parallel
            nc.gpsimd.dma_start(dgt[:],
                                dg_d.rearrange("c t o -> c (t o)"))
            nc.sync.dma_start(xb0[:, 0:20, :], x_d[0][:, 0:20, :])
            nc.sync.dma_start(xb0[:, 20:H, :], x_d[0][:, 20:H, :])

            bv = cp.tile([128, 4], f32)
            nc.gpsimd.dma_start(bv[:], bv_d)
            bias1 = bv[:, 0:1]
            thr1 = bv[:, 1:2]
            bias2 = [bv[:, 2:3], bv[:, 3:4]]
            lhsT_base = cp.tile([128, C_OUT], bf16)
            nc.gpsimd.dma_start(lhsT_base[:], lw_d)

            # warm the PE HAM clock while the first DMAs are in flight
            warm = cp.tile([128, 448], bf16)
            nc.vector.memset(warm[:], 0.0)
            wps = pwps_pool.tile([128, 1024], f32, name="pwps")
            for _ in range(10):
                nc.tensor.matmul(wps[:, 0:448], warm[:, 0:128], warm[:],
                                 start=True, stop=True)

            # rotating engine pickers for drains and output DMAs
            state = {"dr": 0, "dma": 0}

            def drain_op(dst, src, bias, rot):
                state["dr"] += 1
                eng = rot[state["dr"] % len(rot)]
                if eng is nc.scalar:
                    nc.scalar.activation(dst, src, Act.Relu,
                                         bias=bias, scale=1.0)
                else:
                    eng.tensor_scalar(dst, src, bias, 0.0,
                                      Alu.add, Alu.max)

            def dma_engine():
                state["dma"] += 1
                return nc.sync if state["dma"] % 2 else nc.gpsimd

            def emit_dw_pair(img, pi):
                """one PSUM pair of the depthwise conv: 9 taps x <=2 tiles,
                then the keep-stat XY-reduce (VectorE, from PSUM) and the
                paired drain."""
                xb, yb, partdw = img["xb"], img["yb"], img["partdw"]
                pair = img["pairs"][pi]
                ps = dwps_pool.tile([128, 1024], f32, name="dwps")
                for t_idx, (di, dj) in enumerate(TAPS):
                    for k, tt in enumerate(pair):
                        r0 = tt * TILE_ROWS
                        rlo = max(0, r0 + di)
                        rhi = min(H, r0 + TILE_ROWS + di)
                        clo, chi = max(0, dj), min(W, W + dj)
                        rhs = xb[:, rlo:rhi, clo:chi]
                        ps3 = ps[:, k * 512:k * 512 + TN].rearrange(
                            "c (h w) -> c h w", h=TILE_ROWS)
                        out = ps3[:, rlo - di - r0:rhi - di - r0,
                                  clo - dj:chi - dj]
                        nc.tensor.matmul(
                            out, dgt[:, t_idx * 128:(t_idx + 1) * 128], rhs,
                            start=(t_idx == 0), stop=(t_idx == 8))
                npair = len(pair)
                src = ps[:, 0:npair * 512].rearrange(
                    "c (b x) -> c b x", b=npair)[:, :, 0:TN]
                # keep-stat straight from PSUM on VectorE
                nc.vector.tensor_reduce(partdw[:, pi:pi + 1], src,
                                        axis=mybir.AxisListType.XY, op=Alu.max)
                if pi == 2:
                    nc.vector.tensor_reduce(img["mxa"][:], partdw[:, 0:3],
                                            axis=mybir.AxisListType.X,
                                            op=Alu.max)
                c0 = pair[0] * TN
                dst = yb[:, c0:c0 + npair * TN].rearrange(
                    "c (b x) -> c b x", b=npair)
                # dw drains all on Scalar: Vector keeps only the keep-stat
                # reduces + chain, in pipeline order, so PSUM recycling
                # and the chain never queue behind drain bursts
                drain_op(dst, src, bias1, (nc.scalar,))

            def emit_chain(img):
                """keep1 -> masked lhsT halves (all on VectorE)."""
                mx1 = sp.tile([128, 1], f32, name="mx1")
                nc.vector.tensor_max(mx1[:], img["mxa"][:],
                                     img["partdw"][:, 3:4])
                keep1 = sp.tile([128, 1], f32, name="keep1")
                nc.vector.tensor_scalar(keep1[:], mx1[:], thr1,
                                        None, Alu.is_ge)
                for m in range(2):
                    lm = sp.tile([128, 128], bf16, name=f"lhsTm{m}")
                    nc.vector.tensor_scalar(
                        lm[:], lhsT_base[:, m * 128:(m + 1) * 128], keep1[:],
                        None, Alu.mult)
                    img["lhsTm"].append(lm)

            def emit_pw(img, pools, dma_per_pair=False):
                """both 128-out-channel chunks: matmuls + per-pair drains.
                Output DMA per chunk (fewest descriptors) or per pair
                (lowest latency -- used for the last image's tail)."""
                n, yb = img["n"], img["yb"]
                for m in range(2):
                    zrow = z_d[n, m * 128:(m + 1) * 128].rearrange(
                        "c h w -> c (h w)")
                    lhsTm = img["lhsTm"][m]
                    zt = zp.tile([128, HW], bf16, name="zt")
                    for pj, pair in enumerate(PAIRS):
                        pool = pools[pj % len(pools)]
                        ps = pool.tile([128, 1024], f32,
                                       name="dwps" if pool is dwps_pool
                                       else "pwps")
                        for k, tt in enumerate(pair):
                            nc.tensor.matmul(
                                ps[:, k * 512:k * 512 + TN], lhsTm[:],
                                yb[:, tt * TN:(tt + 1) * TN],
                                start=True, stop=True)
                        npair = len(pair)
                        c0 = pair[0] * TN
                        src = ps[:, 0:npair * 512].rearrange(
                            "c (b x) -> c b x", b=npair)[:, :, 0:TN]
                        dst = zt[:, c0:c0 + npair * TN].rearrange(
                            "c (b x) -> c b x", b=npair)
                        drain_op(dst, src, bias2[m], (nc.vector, nc.scalar))
                        if dma_per_pair:
                            dma_engine().dma_start(
                                zrow[:, c0:c0 + npair * TN],
                                zt[:, c0:c0 + npair * TN])
                    if not dma_per_pair:
                        dma_engine().dma_start(zrow[:], zt[:])

            def new_img(n, xb):
                # image 0's pair order follows its two x row-chunks; later
                # images put the single-tile pair FIRST so the last pair
                # (18 matmuls) covers the next image's PSUM-buffer reuse
                if n == 0:
                    # follow image 0's two x row-chunks, but keep a 2-tile
                    # pair last so it covers the next image's PSUM reuse
                    pairs = [PAIRS[0], PAIRS[1], PAIRS[3], PAIRS[2]]
                else:
                    pairs = [PAIRS[3]] + PAIRS[0:3]
                return {"n": n, "xb": xb, "pairs": pairs,
                        "yb": yp.tile([128, HW], bf16, name="ybt"),
                        "partdw": sp.tile([128, 4], f32, name="partdw"),
                        "mxa": sp.tile([128, 1], f32, name="mxa"),
                        "lhsTm": []}

            imgs = [None] * BPC
            imgs[0] = new_img(0, xb0)
            for n in range(BPC):
                img = imgs[n]
                emit_dw_pair(img, 0)
                # prefetch next image's input (one fat DMA, sync queue --
                # FIFO behind image 0's load so it cannot starve it)
                if n + 1 < BPC:
                    xb = xbp.tile([128, H, W], bf16, name="xbt")
                    nc.sync.dma_start(xb[:].rearrange("c h w -> c (h w)"),
                                      x_d[n + 1].rearrange("c h w -> c (h w)"))
                    imgs[n + 1] = new_img(n + 1, xb)
                emit_dw_pair(img, 1)
                emit_dw_pair(img, 2)
                # previous image's pointwise sits between pairs 2 and 3 so
                # the keep-stat reduces of the pairs that gate the next
                # image's PSUM reuse are never queued behind the pw drains
                if n > 0:
                    emit_pw(imgs[n - 1], [pwps_pool])
                emit_dw_pair(img, 3)
                emit_chain(img)
            # cover image 3's keep chain with dummy matmuls, then its pw
            # with both PSUM pools for deeper pipelining
            dps = dwps_pool.tile([128, 1024], f32, name="dwps")
            for _ in range(8):
                nc.tensor.matmul(dps[:, 0:448], warm[:, 0:128], warm[:],
                                 start=True, stop=True)
            emit_pw(imgs[3], [pwps_pool, dwps_pool])

    nc.compile()
    return nc


def _get_nc():
    if "nc" not in _CACHE:
        _CACHE["nc"] = _build()
    return _CACHE["nc"]


def _fold_weights(inputs):
    """Host-side numpy prep of all the small weight algebra."""
    dw_w = np.asarray(inputs["dw_w"], np.float64).reshape(C_IN, 9)
    dw_b = np.asarray(inputs["dw_b"], np.float64)
    g1 = np.asarray(inputs["bn1_g"], np.float64)
    b1 = np.asarray(inputs["bn1_b"], np.float64)
    m1 = np.asarray(inputs["bn1_m"], np.float64)
    v1 = np.asarray(inputs["bn1_v"], np.float64)
    pw_w = np.asarray(inputs["pw_w"], np.float64)
    pw_b = np.asarray(inputs["pw_b"], np.float64)
    g2 = np.asarray(inputs["bn2_g"], np.float64)
    b2 = np.asarray(inputs["bn2_b"], np.float64)
    m2 = np.asarray(inputs["bn2_m"], np.float64)
    v2 = np.asarray(inputs["bn2_v"], np.float64)

    s1 = g1 / np.sqrt(v1 + BN_EPS)
    bias1 = (s1 * (dw_b - m1) + b1).astype(np.float64)
    thr1 = (DW_THRESH - bias1).astype(np.float64)
    dws = dw_w * s1[:, None]                      # [C_IN, 9]
    diags = np.zeros((C_IN, 9, C_IN), np.float32)
    idx = np.arange(C_IN)
    for t, (di, dj) in enumerate(TAPS):
        k = (di + 1) * 3 + (dj + 1)
        diags[idx, t, idx] = dws[:, k]

    s2 = g2 / np.sqrt(v2 + BN_EPS)
    bias2 = (s2 * (pw_b - m2) + b2).astype(np.float64)
    lhsTb = (pw_w * s2[:, None]).T.astype(np.float32)   # [C_IN, C_OUT]

    biasv = np.stack([bias1, thr1, bias2[:C_IN], bias2[C_IN:]],
                     axis=1).astype(np.float32)          # [C_IN, 4]

    import ml_dtypes
    return {
        "diags": np.ascontiguousarray(diags.astype(ml_dtypes.bfloat16)),
        "biasv": np.ascontiguousarray(biasv),
        "lhsTb": np.ascontiguousarray(lhsTb.astype(ml_dtypes.bfloat16)),
    }


def _make_in_maps(inputs):
    import ml_dtypes
    x = np.asarray(inputs["x"]).astype(ml_dtypes.bfloat16)
    folded = _fold_weights(inputs)
    in_maps = []
    for c in range(N_CORES):
        m = {"x": np.ascontiguousarray(x[c * BPC:(c + 1) * BPC])}
        m.update(folded)
        in_maps.append(m)
    return in_maps


def kernel(**inputs):
    from concourse.bass_utils import run_bass_kernel_spmd

    nc = _get_nc()
    in_maps = _make_in_maps(inputs)
    res = run_bass_kernel_spmd(nc, in_maps, core_ids=list(range(N_CORES)))
    _CACHE["last_results"] = res
    z = np.concatenate([np.asarray(res.results[c]["z"])
                        for c in range(N_CORES)], axis=0).astype(np.float32)
    # pw map-cut on host: zero any (n, o) map whose max is below PW_THRESH
    mx = z.max(axis=(2, 3))
    z *= (mx >= PW_THRESH).astype(np.float32)[:, :, None, None]
    return z


# revision 43
# speedup vs baseline: 1.0431x; 1.0431x over previous
"""Trainium2 Bass kernel: DepthSeparableConv2d (dw3x3 + BN + ReLU + map-cut,
pw 1x1 + BN + ReLU + map-cut), data-parallel over batch on 8 NeuronCores.

Host side folds all the small weight algebra (BN scales into conv weights,
pw transpose, biases, the dw-cut threshold) in numpy; the device kernel is
a pure streaming pipeline per core (4 images):

  - depthwise 3x3 conv as 9 diagonal-matmul "taps" on the TensorEngine
    (bf16 diag weights pre-scaled by the BN1 scale, bf16 activations, fp32
    PSUM accumulation); zero padding realized by AP sub-ranges + strided
    PSUM outputs, so the input DMA stays fully contiguous.
  - PSUM tiles are 2-bank [128,1024] pairs; each pair drains in ONE
    Scalar/Vector instruction (relu(psum+bias), bf16), and the dw map-cut
    stat is one VectorE XY-reduce straight from PSUM (keep = max(psum) >=
    4.0 - bias1, threshold folded on host) so the keep chain never waits
    on drains.  keep is folded into the pointwise lhsT halves.
  - pointwise 1x1: 2 chunks of 128 out-channels, bf16 matmuls, drain
    relu(psum+bias2) -> bf16 z per 448-px tile, output DMA per half-chunk.
  - pw map-cut is applied ON HOST (exact max >= 0.001 test in numpy);
    output DMA is bf16 (half traffic), host casts to fp32.
  - schedule: pw(n) is emitted BETWEEN pairs 1 and 2 of dw(n+1), so every
    keep chain and every pw burst is covered by depthwise matmuls; pw3's
    chain is covered by a few dummy matmuls.
"""

import numpy as np

B, C_IN, C_OUT, H, W = 32, 128, 256, 56, 56
N_CORES = 8
BPC = B // N_CORES          # images per core
HW = H * W                  # 3136
TILE_ROWS = 8               # output rows per 448-px sub-tile
NT = H // TILE_ROWS         # 7 sub-tiles per image
TN = TILE_ROWS * W          # 448 pixels per sub-tile
BN_EPS = 1e-5
DW_THRESH = 4.0
PW_THRESH = 0.001

# pairs of 448-px sub-tiles sharing one 2-bank PSUM tile
PAIRS = [(0, 1), (2, 3), (4, 5), (6,)]

# tap order: (0,0) first so the start=True matmul covers the full tile
TAPS = [(0, 0), (-1, 0), (1, 0), (0, -1), (0, 1),
        (-1, -1), (-1, 1), (1, -1), (1, 1)]

_CACHE = {}


def _build():
    import concourse.bacc as bacc
    import concourse.tile as tile
    import concourse.mybir as mybir

    f32 = mybir.dt.float32
    bf16 = mybir.dt.bfloat16
    Alu = mybir.AluOpType
    Act = mybir.ActivationFunctionType

    nc = bacc.Bacc("TRN2", target_bir_lowering=False, debug=False,
                   enable_asserts=False, num_devices=N_CORES)

    x_d = nc.dram_tensor("x", [BPC, C_IN, H, W], bf16, kind="ExternalInput").ap()
    dg_d = nc.dram_tensor("diags", [C_IN, 9, C_IN], bf16, kind="ExternalInput").ap()
    # bias1 / thr1 / bias2-lo / bias2-hi packed as one [C_IN, 4] tensor:
    # a [128,1] vector DMA costs 128 four-byte descriptors, so packing
    # quarters the descriptor load
    bv_d = nc.dram_tensor("biasv", [C_IN, 4], f32, kind="ExternalInput").ap()
    lw_d = nc.dram_tensor("lhsTb", [C_IN, C_OUT], bf16, kind="ExternalInput").ap()
    z_d = nc.dram_tensor("z", [BPC, C_OUT, H, W], bf16, kind="ExternalOutput").ap()

    with tile.TileContext(nc) as tc:
        with tc.tile_pool(name="const", bufs=1) as cp, \
             tc.tile_pool(name="xb", bufs=3) as xbp, \
             tc.tile_pool(name="y", bufs=3) as yp, \
             tc.tile_pool(name="z", bufs=4) as zp, \
             tc.tile_pool(name="small", bufs=8) as sp, \
             tc.tile_pool(name="dwps", bufs=2, space="PSUM") as dwps_pool, \
             tc.tile_pool(name="pwps", bufs=2, space="PSUM") as pwps_pool:

            # ---- startup DMAs: diag weights, then whole image 0, on the
            # sync queue.  One DMA per tensor: the DMA engines are
            # descriptor-rate-bound (~290ns per per-partition line), so
            # fewer, fatter descriptors win ----
            dgt = cp.tile([128, 9 * 128], bf16)
            xb0 = xbp.tile([128, H, W], bf16, name="xbt")
            # dgt on the gpsimd queue, image 0 on the sync queue: their
            # descriptors interleave across the 16 DMA engines in parallel
            nc.gpsimd.dma_start(dgt[:],
                                dg_d.rearrange("c t o -> c (t o)"))
            nc.sync.dma_start(xb0[:, 0:20, :], x_d[0][:, 0:20, :])
            nc.sync.dma_start(xb0[:, 20:H, :], x_d[0][:, 20:H, :])

            bv = cp.tile([128, 4], f32)
            nc.gpsimd.dma_start(bv[:], bv_d)
            bias1 = bv[:, 0:1]
            thr1 = bv[:, 1:2]
            bias2 = [bv[:, 2:3], bv[:, 3:4]]
            lhsT_base = cp.tile([128, C_OUT], bf16)
            nc.gpsimd.dma_start(lhsT_base[:], lw_d)

            # warm the PE HAM clock while the first DMAs are in flight
            warm = cp.tile([128, 448], bf16)
            nc.vector.memset(warm[:], 0.0)
            wps = pwps_pool.tile([128, 1024], f32, name="pwps")
            for _ in range(10):
                nc.tensor.matmul(wps[:, 0:448], warm[:, 0:128], warm[:],
                                 start=True, stop=True)

            # rotating engine pickers for drains and output DMAs
            state = {"dr": 0, "dma": 0}

            def drain_op(dst, src, bias, rot):
                state["dr"] += 1
                eng = rot[state["dr"] % len(rot)]
                if eng is nc.scalar:
                    nc.scalar.activation(dst, src, Act.Relu,
                                         bias=bias, scale=1.0)
                else:
                    eng.tensor_scalar(dst, src, bias, 0.0,
                                      Alu.add, Alu.max)

            def dma_engine():
                # all output DMAs on the sync queue: it is idle after the
                # startup loads, while gpsimd's queue carries scheduler
                # waits that delay its DMA issues by many microseconds
                return nc.sync

            def emit_dw_pair(img, pi):
                """one PSUM pair of the depthwise conv: 9 taps x <=2 tiles,
                then the keep-stat XY-reduce (VectorE, from PSUM) and the
                paired drain."""
                xb, yb, partdw = img["xb"], img["yb"], img["partdw"]
                pair = img["pairs"][pi]
                ps = dwps_pool.tile([128, 1024], f32, name="dwps")
                for t_idx, (di, dj) in enumerate(TAPS):
                    for k, tt in enumerate(pair):
                        r0 = tt * TILE_ROWS
                        rlo = max(0, r0 + di)
                        rhi = min(H, r0 + TILE_ROWS + di)
                        clo, chi = max(0, dj), min(W, W + dj)
                        rhs = xb[:, rlo:rhi, clo:chi]
                        ps3 = ps[:, k * 512:k * 512 + TN].rearrange(
                            "c (h w) -> c h w", h=TILE_ROWS)
                        out = ps3[:, rlo - di - r0:rhi - di - r0,
                                  clo - dj:chi - dj]
                        nc.tensor.matmul(
                            out, dgt[:, t_idx * 128:(t_idx + 1) * 128], rhs,
                            start=(t_idx == 0), stop=(t_idx == 8))
                npair = len(pair)
                src = ps[:, 0:npair * 512].rearrange(
                    "c (b x) -> c b x", b=npair)[:, :, 0:TN]
                c0 = pair[0] * TN
                dst = yb[:, c0:c0 + npair * TN].rearrange(
                    "c (b x) -> c b x", b=npair)
                # dw drains all on Scalar: the drain is then the PSUM
                # pair's ONLY reader, so the buffer recycles fast
                drain_op(dst, src, bias1, (nc.scalar,))
                # keep-stat from the drained y (VectorE): max(relu(y)) is
                # compared against the raw 4.0 threshold later
                nc.vector.tensor_reduce(partdw[:, pi:pi + 1],
                                        yb[:, c0:c0 + npair * TN],
                                        axis=mybir.AxisListType.X, op=Alu.max)
                if pi == 2:
                    nc.vector.tensor_reduce(img["mxa"][:], partdw[:, 0:3],
                                            axis=mybir.AxisListType.X,
                                            op=Alu.max)

            def emit_chain(img):
                """keep1 -> masked lhsT halves (all on VectorE)."""
                mx1 = sp.tile([128, 1], f32, name="mx1")
                nc.vector.tensor_max(mx1[:], img["mxa"][:],
                                     img["partdw"][:, 3:4])
                keep1 = sp.tile([128, 1], f32, name="keep1")
                nc.vector.tensor_scalar(keep1[:], mx1[:], float(DW_THRESH),
                                        None, Alu.is_ge)
                for m in range(2):
                    lm = sp.tile([128, 128], bf16, name=f"lhsTm{m}")
                    nc.vector.tensor_scalar(
                        lm[:], lhsT_base[:, m * 128:(m + 1) * 128], keep1[:],
                        None, Alu.mult)
                    img["lhsTm"].append(lm)

            def emit_pw(img, pools, dma_per_pair=False):
                """both 128-out-channel chunks: matmuls + per-pair drains.
                Output DMA per chunk (fewest descriptors) or per pair
                (lowest latency -- used for the last image's tail)."""
                n, yb = img["n"], img["yb"]
                for m in range(2):
                    zrow = z_d[n, m * 128:(m + 1) * 128].rearrange(
                        "c h w -> c (h w)")
                    lhsTm = img["lhsTm"][m]
                    zt = zp.tile([128, HW], bf16, name="zt")
                    for pj, pair in enumerate(PAIRS):
                        pool = pools[pj % len(pools)]
                        ps = pool.tile([128, 1024], f32,
                                       name="dwps" if pool is dwps_pool
                                       else "pwps")
                        for k, tt in enumerate(pair):
                            nc.tensor.matmul(
                                ps[:, k * 512:k * 512 + TN], lhsTm[:],
                                yb[:, tt * TN:(tt + 1) * TN],
                                start=True, stop=True)
                        npair = len(pair)
                        c0 = pair[0] * TN
                        src = ps[:, 0:npair * 512].rearrange(
                            "c (b x) -> c b x", b=npair)[:, :, 0:TN]
                        dst = zt[:, c0:c0 + npair * TN].rearrange(
                            "c (b x) -> c b x", b=npair)
                        drain_op(dst, src, bias2[m], (nc.vector, nc.scalar))
                        if dma_per_pair:
                            dma_engine().dma_start(
                                zrow[:, c0:c0 + npair * TN],
                                zt[:, c0:c0 + npair * TN])
                    if not dma_per_pair:
                        dma_engine().dma_start(zrow[:], zt[:])

            def new_img(n, xb):
                # image 0's pair order follows its two x row-chunks; later
                # images put the single-tile pair FIRST so the last pair
                # (18 matmuls) covers the next image's PSUM-buffer reuse
                if n == 0:
                    # follow image 0's two x row-chunks, but keep a 2-tile
                    # pair last so it covers the next image's PSUM reuse
                    pairs = [PAIRS[0], PAIRS[1], PAIRS[3], PAIRS[2]]
                else:
                    pairs = [PAIRS[3]] + PAIRS[0:3]
                return {"n": n, "xb": xb, "pairs": pairs,
                        "yb": yp.tile([128, HW], bf16, name="ybt"),
                        "partdw": sp.tile([128, 4], f32, name="partdw"),
                        "mxa": sp.tile([128, 1], f32, name="mxa"),
                        "lhsTm": []}

            imgs = [None] * BPC
            imgs[0] = new_img(0, xb0)
            for n in range(BPC):
                img = imgs[n]
                emit_dw_pair(img, 0)
                # prefetch next image's input (one fat DMA, sync queue --
                # FIFO behind image 0's load so it cannot starve it)
                if n + 1 < BPC:
                    xb = xbp.tile([128, H, W], bf16, name="xbt")
                    nc.sync.dma_start(xb[:].rearrange("c h w -> c (h w)"),
                                      x_d[n + 1].rearrange("c h w -> c (h w)"))
                    imgs[n + 1] = new_img(n + 1, xb)
                emit_dw_pair(img, 1)
                emit_dw_pair(img, 2)
                # previous image's pointwise sits between pairs 2 and 3 so
                # the keep-stat reduces of the pairs that gate the next
                # image's PSUM reuse are never queued behind the pw drains
                if n > 0:
                    emit_pw(imgs[n - 1], [pwps_pool])
                emit_dw_pair(img, 3)
                emit_chain(img)
            # cover image 3's keep chain with dummy matmuls, then its pw
            # with both PSUM pools for deeper pipelining
            dps = dwps_pool.tile([128, 1024], f32, name="dwps")
            for _ in range(16):
                nc.tensor.matmul(dps[:, 0:448], warm[:, 0:128], warm[:],
                                 start=True, stop=True)
            emit_pw(imgs[3], [pwps_pool, dwps_pool])

    nc.compile()
    return nc


def _get_nc():
    if "nc" not in _CACHE:
        _CACHE["nc"] = _build()
    return _CACHE["nc"]


def _fold_weights(inputs):
    """Host-side numpy prep of all the small weight algebra."""
    dw_w = np.asarray(inputs["dw_w"], np.float64).reshape(C_IN, 9)
    dw_b = np.asarray(inputs["dw_b"], np.float64)
    g1 = np.asarray(inputs["bn1_g"], np.float64)
    b1 = np.asarray(inputs["bn1_b"], np.float64)
    m1 = np.asarray(inputs["bn1_m"], np.float64)
    v1 = np.asarray(inputs["bn1_v"], np.float64)
    pw_w = np.asarray(inputs["pw_w"], np.float64)
    pw_b = np.asarray(inputs["pw_b"], np.float64)
    g2 = np.asarray(inputs["bn2_g"], np.float64)
    b2 = np.asarray(inputs["bn2_b"], np.float64)
    m2 = np.asarray(inputs["bn2_m"], np.float64)
    v2 = np.asarray(inputs["bn2_v"], np.float64)

    s1 = g1 / np.sqrt(v1 + BN_EPS)
    bias1 = (s1 * (dw_b - m1) + b1).astype(np.float64)
    thr1 = (DW_THRESH - bias1).astype(np.float64)
    dws = dw_w * s1[:, None]                      # [C_IN, 9]
    diags = np.zeros((C_IN, 9, C_IN), np.float32)
    idx = np.arange(C_IN)
    for t, (di, dj) in enumerate(TAPS):
        k = (di + 1) * 3 + (dj + 1)
        diags[idx, t, idx] = dws[:, k]

    s2 = g2 / np.sqrt(v2 + BN_EPS)
    bias2 = (s2 * (pw_b - m2) + b2).astype(np.float64)
    lhsTb = (pw_w * s2[:, None]).T.astype(np.float32)   # [C_IN, C_OUT]

    biasv = np.stack([bias1, thr1, bias2[:C_IN], bias2[C_IN:]],
                     axis=1).astype(np.float32)          # [C_IN, 4]

    import ml_dtypes
    return {
        "diags": np.ascontiguousarray(diags.astype(ml_dtypes.bfloat16)),
        "biasv": np.ascontiguousarray(biasv),
        "lhsTb": np.ascontiguousarray(lhsTb.astype(ml_dtypes.bfloat16)),
    }


def _make_in_maps(inputs):
    import ml_dtypes
    x = np.asarray(inputs["x"]).astype(ml_dtypes.bfloat16)
    folded = _fold_weights(inputs)
    in_maps = []
    for c in range(N_CORES):
        m = {"x": np.ascontiguousarray(x[c * BPC:(c + 1) * BPC])}
        m.update(folded)
        in_maps.append(m)
    return in_maps


def kernel(**inputs):
    from concourse.bass_utils import run_bass_kernel_spmd

    nc = _get_nc()
    in_maps = _make_in_maps(inputs)
    res = run_bass_kernel_spmd(nc, in_maps, core_ids=list(range(N_CORES)))
    _CACHE["last_results"] = res
    z = np.concatenate([np.asarray(res.results[c]["z"])
                        for c in range(N_CORES)], axis=0).astype(np.float32)
    # pw map-cut on host: zero any (n, o) map whose max is below PW_THRESH
    mx = z.max(axis=(2, 3))
    z *= (mx >= PW_THRESH).astype(np.float32)[:, :, None, None]
    return z
